# revision 17
# baseline (speedup 1.0000x reference)
"""Grouped self-attention (B=2, S=2048, D=1024, H=16, hd=64) on 8 trn2 cores.

Sharding: core c = b*4 + g handles batch b, heads [4g, 4g+4).

Key simplification: the reference's RoPE indexes its cos/sin cache by the
BATCH dim and uses neg_half = [t_first, -t_second], so rope(t)[b,s,h,d] =
t * (cos(b*th[d%32]) + sign(d)*sin(b*th[d%32])) — a pure per-(b,d) scale
that folds into rows of Wq/Wk on the host. The device kernel is then just
QKV projection + softmax attention.

v5 design (evolved via NTFF profiling):
- all matmul operands bf16 (fp32r streams at ~half rate), PSUM accum f32.
- phase 2 is ACT-bound: 128 exp tiles [128,1024] at ~1.1us each, ACT
  saturated back-to-back. Everything else aims to start that pipeline
  early and keep it unbroken.
- K stored as 4 zero-padded per-head tiles ktz[h] (real rows in the
  head's pair slot, zeros elsewhere) so scores matmuls run with a full
  128 contraction (64-contraction matmuls measured ~2.5x slower);
  Q keeps the packed pair layout (padded K rows zero the other head).
- PV: out[q,d] = et_chunk.T @ V_aug (ones column = softmax denominator).
  4 accumulation slices share a PSUM bank and start=True zeroes a whole
  bank, so banks are pre-zeroed via DVE memset and PV accumulates with
  start=False.
- DMA uses fat lines only (>=4KB per partition): xt as 8 kc-chunks of
  [128,2048], each weight matrix as a single [128,2048] transfer
  (small-line layouts measured at ~130GB/s vs ~290GB/s).
- the kt0 projection is 4-way chunk-interleaved (4 concurrent PSUM bank
  accumulators) so it tracks DMA arrival and finishes with the last xt
  chunk; first exp fires right after qt0 + V.
- ACT does exp only; PSUM reads (copies/normalize) on DVE; ktz padding
  memsets on GpSimd; output DMA'd per 128-row tile as its last head
  normalizes, hiding the writeback tail.
"""

import numpy as np
from contextlib import ExitStack

import ml_dtypes
import concourse.bass as bass
import concourse.bacc as bacc
import concourse.tile as tile
from concourse import mybir
from concourse.bass_utils import run_bass_kernel_spmd

F32 = mybir.dt.float32
BF16 = mybir.dt.bfloat16
EXP = mybir.ActivationFunctionType.Exp

B, S, D, H, HD = 2, 2048, 1024, 16, 64
NCORES = 8

_CACHE = {}


def _build_nc():
    nc = bacc.Bacc("TRN2", target_bir_lowering=False, debug=False)
    # xt piece half*8+kc = x[b].T[kc*128:(kc+1)*128, half*1024:(half+1)*1024]
    # (2KB lines measured at ~357GB/s; s-half-major order lets projections
    # and V start before the full tensor lands)
    xt_d = nc.declare_dram_parameter("xt", [16, 128, 1024], BF16, isOutput=False)
    # weights laid out [128, 8, 256] so one fat DMA fills the SBUF tile
    wqt_d = nc.declare_dram_parameter("wqt", [128, 8 * 256], BF16, isOutput=False)
    wkt_d = nc.declare_dram_parameter("wkt", [128, 8 * 256], BF16, isOutput=False)
    wvt_d = nc.declare_dram_parameter("wvt", [128, 8 * 256], BF16, isOutput=False)
    mb_d = nc.declare_dram_parameter("maskb", [128, 16], F32, isOutput=False)
    out_d = nc.declare_dram_parameter("out", [S, 256], F32, isOutput=True)

    with tile.TileContext(nc) as tc, ExitStack() as ctx:
        const = ctx.enter_context(tc.tile_pool(name="const", bufs=1))
        xpool = ctx.enter_context(tc.tile_pool(name="x", bufs=1))
        wpool = ctx.enter_context(tc.tile_pool(name="w", bufs=1))
        qkpool = ctx.enter_context(tc.tile_pool(name="qk", bufs=1))
        vpool = ctx.enter_context(tc.tile_pool(name="v", bufs=1))
        opool = ctx.enter_context(tc.tile_pool(name="o", bufs=1))
        epool = ctx.enter_context(tc.tile_pool(name="et", bufs=3))
        small = ctx.enter_context(tc.tile_pool(name="small", bufs=4))
        scp = ctx.enter_context(tc.tile_pool(name="scp", bufs=2, space="PSUM"))
        pvp = ctx.enter_context(tc.tile_pool(name="pvp", bufs=2, space="PSUM"))

        mb = const.tile([128, 16], F32)
        nc.sync.dma_start(mb[:], mb_d[:])
        wk = wpool.tile([128, 8 * 256], BF16, tag="wk")
        wq = wpool.tile([128, 8 * 256], BF16, tag="wq")
        wv = wpool.tile([128, 8 * 256], BF16, tag="wv")
        nc.sync.dma_start(wk[:], wkt_d[:])
        nc.sync.dma_start(wq[:], wqt_d[:])
        xt = xpool.tile([128, 8 * S], BF16)
        for kc in range(8):
            nc.sync.dma_start(xt[:, kc * S: kc * S + 1024], xt_d[kc])
        nc.sync.dma_start(wv[:], wvt_d[:])
        for kc in range(8):
            nc.sync.dma_start(xt[:, kc * S + 1024: kc * S + 2048], xt_d[8 + kc])

        qt = [qkpool.tile([128, S], BF16, tag=f"qt{p}", name=f"qt{p}")
              for p in range(2)]
        ktz = [qkpool.tile([128, S], BF16, tag=f"ktz{h}", name=f"ktz{h}")
               for h in range(4)]
        # zero the padding rows once (GpSimd: SBUF-only engine, keeps DVE free)
        for h in range(4):
            lo, hi = (64, 128) if h % 2 == 0 else (0, 64)
            nc.gpsimd.memset(ktz[h][lo:hi, :], 0.0)

        v_sb = vpool.tile([128, 16, 4, 65], BF16)
        nc.gpsimd.memset(v_sb[:, :, :, 64:65], 1.0)

        def k_copies(pair, nb, ps):
            sl = slice(nb * 512, (nb + 1) * 512)
            nc.vector.tensor_copy(ktz[2 * pair][0:64, sl], ps[0:64, 0:512])
            nc.vector.tensor_copy(ktz[2 * pair + 1][64:128, sl], ps[64:128, 0:512])

        def proj_q(pair, nb):
            ps = scp.tile([128, 1024], F32, tag="sc")
            for kc in range(8):
                lo = kc * 256 + pair * 128
                nc.tensor.matmul(
                    ps[:, 0:512],
                    lhsT=wq[:, lo:lo + 128],
                    rhs=xt[:, kc * S + nb * 512: kc * S + nb * 512 + 512],
                    start=(kc == 0), stop=(kc == 7))
            nc.vector.tensor_copy(qt[pair][:, nb * 512:(nb + 1) * 512],
                                  ps[:, 0:512])

        def proj_k(pair, nb):
            ps = scp.tile([128, 1024], F32, tag="sc")
            for kc in range(8):
                lo = kc * 256 + pair * 128
                nc.tensor.matmul(
                    ps[:, 0:512],
                    lhsT=wk[:, lo:lo + 128],
                    rhs=xt[:, kc * S + nb * 512: kc * S + nb * 512 + 512],
                    start=(kc == 0), stop=(kc == 7))
            k_copies(pair, nb, ps)

        def proj_v(m):
            pv = scp.tile([128, 1024], F32, tag="sc")
            for kc in range(8):
                nc.tensor.matmul(
                    pv[:, 0:256],
                    lhsT=xt[:, kc * S + m * 128: kc * S + m * 128 + 128],
                    rhs=wv[:, kc * 256:(kc + 1) * 256],
                    start=(kc == 0), stop=(kc == 7))
            nc.vector.tensor_copy(v_sb[:, m, :, 0:64], pv[:, 0:256])

        # ---- phase 1 ----
        # Per s-half: kt0 and qt0 nb-groups chunk-interleaved across 4 PSUM
        # banks (2 sc tiles x 2 bank-halves) tracking xt DMA arrival, then
        # that half's V while the next half streams in.
        def qk_half(half):
            nb0 = 2 * half
            t1 = scp.tile([128, 1024], F32, tag="sc")
            t2 = scp.tile([128, 1024], F32, tag="sc")
            slots = [t1[:, 0:512], t1[:, 512:1024],
                     t2[:, 0:512], t2[:, 512:1024]]
            for kc in range(8):
                for i, (wtile, nb) in enumerate(
                        ((wk, nb0), (wk, nb0 + 1), (wq, nb0), (wq, nb0 + 1))):
                    nc.tensor.matmul(
                        slots[i],
                        lhsT=wtile[:, kc * 256: kc * 256 + 128],
                        rhs=xt[:, kc * S + nb * 512: kc * S + nb * 512 + 512],
                        start=(kc == 0), stop=(kc == 7))
            for i, nb in ((0, nb0), (1, nb0 + 1)):
                sl = slice(nb * 512, (nb + 1) * 512)
                nc.vector.tensor_copy(ktz[0][0:64, sl], slots[i][0:64, :])
                nc.vector.tensor_copy(ktz[1][64:128, sl], slots[i][64:128, :])
            for i, nb in ((2, nb0), (3, nb0 + 1)):
                nc.vector.tensor_copy(qt[0][:, nb * 512:(nb + 1) * 512],
                                      slots[i][:, :])

        qk_half(0)
        for m in range(8):
            proj_v(m)
        qk_half(1)
        for m in range(8, 16):
            proj_v(m)

        # pair-1 projections are deferred into phase-2 group boundaries
        deferred = [lambda nb=nb: proj_k(1, nb) for nb in range(4)]
        deferred += [lambda nb=nb: proj_q(1, nb) for nb in range(4)]

        # ---- phase 2 ----
        ost = opool.tile([128, 16, 256], F32)

        def attn_group(qh, h, last_head):
            pair = h // 2
            # 4 accumulation slices share a PSUM bank and a matmul with
            # start=True zeroes the WHOLE bank, so pre-zero via DVE and
            # accumulate with start=False on every PV matmul.
            pva = pvp.tile([128, 4, 65], F32, tag="pva")
            pvb = pvp.tile([128, 4, 65], F32, tag="pvb")
            nc.vector.memset(pva[:], 0.0)
            nc.vector.memset(pvb[:], 0.0)
            for kb in range(16):
                ps = scp.tile([128, 1024], F32, tag="sc")
                for j in range(2):
                    q0 = qh * 1024 + j * 512
                    nc.tensor.matmul(
                        ps[:, j * 512:(j + 1) * 512],
                        lhsT=ktz[h][:, kb * 128:(kb + 1) * 128],
                        rhs=qt[pair][:, q0:q0 + 512],
                        start=True, stop=True)
                et = epool.tile([128, 1024], BF16)
                nc.scalar.activation(et[:], ps[:], EXP,
                                     bias=mb[:, kb:kb + 1], scale=0.125)
                for t in range(8):
                    dst = pva if t < 4 else pvb
                    nc.tensor.matmul(
                        dst[:, t % 4, :],
                        lhsT=et[:, t * 128:(t + 1) * 128],
                        rhs=v_sb[:, kb, h, :],
                        start=False, stop=(kb == 15),
                        skip_group_check=True)
            rca = small.tile([128, 4, 1], F32, tag="rca")
            rcb = small.tile([128, 4, 1], F32, tag="rcb")
            nc.vector.reciprocal(rca[:], pva[:, :, 64:65])
            nc.vector.reciprocal(rcb[:], pvb[:, :, 64:65])
            for t in range(8):
                src = pva if t < 4 else pvb
                rc = rca if t < 4 else rcb
                m = qh * 8 + t
                nc.vector.tensor_scalar_mul(
                    ost[:, m, h * 64:h * 64 + 64],
                    src[:, t % 4, 0:64], rc[:, t % 4, :])
                if last_head:
                    nc.sync.dma_start(out_d[m * 128:(m + 1) * 128, :],
                                      ost[:, m, :])

        # pair-0 groups first (pair-1 projections slot in at boundaries)
        groups = [(0, 0), (0, 1), (1, 0), (1, 1),
                  (0, 2), (0, 3), (1, 2), (1, 3)]
        for gi, (qh, h) in enumerate(groups):
            attn_group(qh, h, last_head=(h == 3))
            if gi < 4:
                deferred.pop(0)()
                deferred.pop(0)()
    nc.compile()
    return nc


def _host_prep(x, attention_mask, Wq, Wk, Wv):
    x = np.asarray(x, dtype=np.float32)
    mask = np.asarray(attention_mask)
    Wq = np.asarray(Wq, dtype=np.float32)
    Wk = np.asarray(Wk, dtype=np.float32)
    Wv = np.asarray(Wv, dtype=np.float32)
    bf16 = ml_dtypes.bfloat16

    # rope fold: c_eff[b, d] = cos(b*th[d%32]) + sign(d)*sin(b*th[d%32])
    j = np.arange(0, HD, 2, dtype=np.float64) / HD          # [32]
    theta = 1.0 / (10000.0 ** j)                            # [32]
    dd = np.arange(HD)
    sign = np.where(dd < 32, 1.0, -1.0)

    def wlayout(wt_cols):  # [1024(k), 256] -> [128, 8*256] (partition-major)
        return np.ascontiguousarray(
            wt_cols.reshape(8, 128, 256).transpose(1, 0, 2).reshape(128, 8 * 256))

    in_maps = []
    wvt_full = np.ascontiguousarray(Wv.T).astype(bf16)      # [1024,1024]
    for b in range(B):
        ang = b * theta                                     # [32]
        ce = np.cos(ang[dd % 32]) + sign * np.sin(ang[dd % 32])  # [64]
        ccol = np.tile(ce, H).astype(np.float32)            # [1024]
        wqt_full = np.ascontiguousarray((Wq * ccol[:, None]).T).astype(bf16)
        wkt_full = np.ascontiguousarray((Wk * ccol[:, None]).T).astype(bf16)
        xtT = np.ascontiguousarray(x[b].T).astype(bf16)     # [1024, 2048]
        # [16 pieces, 128, 1024]: piece half*8+kc = xtT[kc-chunk, s-half]
        xt = np.ascontiguousarray(
            xtT.reshape(8, 128, 2, 1024).transpose(2, 0, 1, 3)).reshape(
            16, 128, 1024)
        maskb = np.ascontiguousarray(
            ((mask[b].astype(np.float32) - 1.0) * 30000.0).reshape(16, 128).T)
        for g in range(4):
            cols = slice(g * 256, (g + 1) * 256)
            in_maps.append({
                "xt": xt,
                "wqt": wlayout(wqt_full[:, cols]),
                "wkt": wlayout(wkt_full[:, cols]),
                "wvt": wlayout(wvt_full[:, cols]),
                "maskb": maskb,
            })
    return in_maps


def _get_nc():
    if "nc" not in _CACHE:
        _CACHE["nc"] = _build_nc()
    return _CACHE["nc"]


def kernel(x, attention_mask, Wq, Wk, Wv, **extra_kwargs):
    nc = _get_nc()
    in_maps = _host_prep(x, attention_mask, Wq, Wk, Wv)
    res = run_bass_kernel_spmd(nc, in_maps, list(range(NCORES))).results
    out = np.empty((B, S, D), dtype=np.float32)
    for c in range(NCORES):
        b, g = divmod(c, 4)
        out[b, :, g * 256:(g + 1) * 256] = res[c]["out"]
    return out


# revision 24
# speedup vs baseline: 1.1240x; 1.1240x over previous
"""Grouped self-attention (B=2, S=2048, D=1024, H=16, hd=64) on 8 trn2 cores.

Sharding: core c = b*4 + g handles batch b, heads [4g, 4g+4).

Key simplification: the reference's RoPE indexes its cos/sin cache by the
BATCH dim and uses neg_half = [t_first, -t_second], so rope(t)[b,s,h,d] =
t * (cos(b*th[d%32]) + sign(d)*sin(b*th[d%32])) — a pure per-(b,d) scale
that folds into rows of Wq/Wk on the host. The device kernel is then just
QKV projection + softmax attention.

v5 design (evolved via NTFF profiling):
- all matmul operands bf16 (fp32r streams at ~half rate), PSUM accum f32.
- phase 2 is ACT-bound: 128 exp tiles [128,1024] at ~1.1us each, ACT
  saturated back-to-back. Everything else aims to start that pipeline
  early and keep it unbroken.
- K stored as 4 zero-padded per-head tiles ktz[h] (real rows in the
  head's pair slot, zeros elsewhere) so scores matmuls run with a full
  128 contraction (64-contraction matmuls measured ~2.5x slower);
  Q keeps the packed pair layout (padded K rows zero the other head).
- PV: out[q,d] = et_chunk.T @ V_aug (ones column = softmax denominator).
  4 accumulation slices share a PSUM bank and start=True zeroes a whole
  bank, so banks are pre-zeroed via DVE memset and PV accumulates with
  start=False.
- DMA uses fat lines only (>=4KB per partition): xt as 8 kc-chunks of
  [128,2048], each weight matrix as a single [128,2048] transfer
  (small-line layouts measured at ~130GB/s vs ~290GB/s).
- the kt0 projection is 4-way chunk-interleaved (4 concurrent PSUM bank
  accumulators) so it tracks DMA arrival and finishes with the last xt
  chunk; first exp fires right after qt0 + V.
- ACT does exp only; PSUM reads (copies/normalize) on DVE; ktz padding
  memsets on GpSimd; output DMA'd per 128-row tile as its last head
  normalizes, hiding the writeback tail.
"""

import numpy as np
from contextlib import ExitStack

import ml_dtypes
import concourse.bass as bass
import concourse.bacc as bacc
import concourse.tile as tile
from concourse import mybir
from concourse.bass_utils import run_bass_kernel_spmd

F32 = mybir.dt.float32
BF16 = mybir.dt.bfloat16
EXP = mybir.ActivationFunctionType.Exp

B, S, D, H, HD = 2, 2048, 1024, 16, 64
NCORES = 8

_CACHE = {}


def _build_nc():
    nc = bacc.Bacc("TRN2", target_bir_lowering=False, debug=False)
    # xt piece half*8+kc = x[b].T[kc*128:(kc+1)*128, half*1024:(half+1)*1024]
    # (2KB lines measured at ~357GB/s; s-half-major order lets projections
    # and V start before the full tensor lands)
    xt_d = nc.declare_dram_parameter("xt", [16, 128, 1024], BF16, isOutput=False)
    # weights laid out [128, 8, 256] so one fat DMA fills the SBUF tile
    wqt_d = nc.declare_dram_parameter("wqt", [128, 8 * 256], BF16, isOutput=False)
    wkt_d = nc.declare_dram_parameter("wkt", [128, 8 * 256], BF16, isOutput=False)
    wvt_d = nc.declare_dram_parameter("wvt", [128, 8 * 256], BF16, isOutput=False)
    mb_d = nc.declare_dram_parameter("maskb", [128, 16], F32, isOutput=False)
    out_d = nc.declare_dram_parameter("out", [S, 256], F32, isOutput=True)

    with tile.TileContext(nc) as tc, ExitStack() as ctx:
        const = ctx.enter_context(tc.tile_pool(name="const", bufs=1))
        xpool = ctx.enter_context(tc.tile_pool(name="x", bufs=1))
        wpool = ctx.enter_context(tc.tile_pool(name="w", bufs=1))
        qkpool = ctx.enter_context(tc.tile_pool(name="qk", bufs=1))
        vpool = ctx.enter_context(tc.tile_pool(name="v", bufs=1))
        opool = ctx.enter_context(tc.tile_pool(name="o", bufs=1))
        epool = ctx.enter_context(tc.tile_pool(name="et", bufs=6))
        small = ctx.enter_context(tc.tile_pool(name="small", bufs=4))
        scp = ctx.enter_context(tc.tile_pool(name="scp", bufs=2, space="PSUM"))
        pvp = ctx.enter_context(tc.tile_pool(name="pvp", bufs=1, space="PSUM"))
        pjp = ctx.enter_context(tc.tile_pool(name="pjp", bufs=2, space="PSUM"))

        mb = const.tile([128, 16], F32)
        nc.sync.dma_start(mb[:], mb_d[:])
        wk = wpool.tile([128, 8 * 256], BF16, tag="wk")
        wq = wpool.tile([128, 8 * 256], BF16, tag="wq")
        wv = wpool.tile([128, 8 * 256], BF16, tag="wv")
        nc.sync.dma_start(wk[:], wkt_d[:])
        nc.sync.dma_start(wq[:], wqt_d[:])
        xt = xpool.tile([128, 8 * S], BF16)
        for kc in range(8):
            nc.sync.dma_start(xt[:, kc * S: kc * S + 1024], xt_d[kc])
        nc.sync.dma_start(wv[:], wvt_d[:])
        for kc in range(8):
            nc.sync.dma_start(xt[:, kc * S + 1024: kc * S + 2048], xt_d[8 + kc])

        qt = [qkpool.tile([128, S], BF16, tag=f"qt{p}", name=f"qt{p}")
              for p in range(2)]
        ktz = [qkpool.tile([128, S], BF16, tag=f"ktz{h}", name=f"ktz{h}")
               for h in range(4)]
        # zero the padding rows once (GpSimd: SBUF-only engine, keeps DVE free)
        for h in range(4):
            lo, hi = (64, 128) if h % 2 == 0 else (0, 64)
            nc.gpsimd.memset(ktz[h][lo:hi, :], 0.0)

        v_sb = vpool.tile([128, 16, 4, 65], BF16)
        nc.gpsimd.memset(v_sb[:, :, :, 64:65], 1.0)

        # prefix copies alternate DVE / ACT (ACT is idle until the first exp)
        _ceng = [0]

        def copy_eng():
            _ceng[0] ^= 1
            return nc.vector if _ceng[0] else nc.scalar

        def prefix_copy(dst, src):
            eng = copy_eng()
            if eng is nc.vector:
                eng.tensor_copy(dst, src)
            else:
                eng.copy(dst, src)

        def k_copies(pair, nb, ps, eng=None):
            sl = slice(nb * 512, (nb + 1) * 512)
            prefix_copy(ktz[2 * pair][0:64, sl], ps[0:64, 0:512])
            prefix_copy(ktz[2 * pair + 1][64:128, sl], ps[64:128, 0:512])

        def proj_v(m):
            pv = scp.tile([128, 1024], F32, tag="sc")
            for kc in range(8):
                nc.tensor.matmul(
                    pv[:, 0:256],
                    lhsT=xt[:, kc * S + m * 128: kc * S + m * 128 + 128],
                    rhs=wv[:, kc * 256:(kc + 1) * 256],
                    start=(kc == 0), stop=(kc == 7))
            prefix_copy(v_sb[:, m, :, 0:64], pv[:, 0:256])

        # ---- phase 1 ----
        # Per s-half: kt0 and qt0 nb-groups chunk-interleaved across 4 PSUM
        # banks (2 sc tiles x 2 bank-halves) tracking xt DMA arrival, then
        # that half's V while the next half streams in.
        def qk_half(half):
            nb0 = 2 * half
            t1 = scp.tile([128, 1024], F32, tag="sc")
            t2 = scp.tile([128, 1024], F32, tag="sc")
            slots = [t1[:, 0:512], t1[:, 512:1024],
                     t2[:, 0:512], t2[:, 512:1024]]
            for kc in range(8):
                for i, (wtile, nb) in enumerate(
                        ((wk, nb0), (wk, nb0 + 1), (wq, nb0), (wq, nb0 + 1))):
                    nc.tensor.matmul(
                        slots[i],
                        lhsT=wtile[:, kc * 256: kc * 256 + 128],
                        rhs=xt[:, kc * S + nb * 512: kc * S + nb * 512 + 512],
                        start=(kc == 0), stop=(kc == 7))
            for i, nb in ((0, nb0), (1, nb0 + 1)):
                sl = slice(nb * 512, (nb + 1) * 512)
                prefix_copy(ktz[0][0:64, sl], slots[i][0:64, :])
                prefix_copy(ktz[1][64:128, sl], slots[i][64:128, :])
            for i, nb in ((2, nb0), (3, nb0 + 1)):
                prefix_copy(qt[0][:, nb * 512:(nb + 1) * 512], slots[i][:, :])

        # qk_half(1) last: group00's first 8 score chunks only need ktz nb0-1,
        # so qk_half(1)'s trailing copies hide under group00's early iterations
        qk_half(0)
        for m in range(16):
            proj_v(m)
        qk_half(1)

        # pair-1 projections: one matmul per phase-2 kb iteration through a
        # dedicated 2-bank PSUM tag; copies on DVE (ACT is saturated then)
        pieces = [(wk, nb, "k") for nb in range(4)]
        pieces += [(wq, nb, "q") for nb in range(4)]
        pstate = {"pi": 0, "kc": 0, "ps": None}

        def emit_proj_step():
            if pstate["pi"] >= len(pieces):
                return
            wtile, nb, kind = pieces[pstate["pi"]]
            kc = pstate["kc"]
            if kc == 0:
                pstate["ps"] = pjp.tile([128, 512], F32, tag="pj",
                                        name=f"pj{pstate['pi']}")
            lo = kc * 256 + 128
            nc.tensor.matmul(
                pstate["ps"][:],
                lhsT=wtile[:, lo:lo + 128],
                rhs=xt[:, kc * S + nb * 512: kc * S + nb * 512 + 512],
                start=(kc == 0), stop=(kc == 7))
            pstate["kc"] += 1
            if pstate["kc"] == 8:
                ps = pstate["ps"]
                sl = slice(nb * 512, (nb + 1) * 512)
                if kind == "k":
                    nc.vector.tensor_copy(ktz[2][0:64, sl], ps[0:64, :])
                    nc.vector.tensor_copy(ktz[3][64:128, sl], ps[64:128, :])
                else:
                    nc.vector.tensor_copy(qt[1][:, sl], ps[:, :])
                pstate["kc"] = 0
                pstate["pi"] += 1

        # ---- phase 2 ----
        ost = opool.tile([128, 16, 256], F32)

        LAG = 3  # PV trails exp by 3 iterations so pvp bufs=1 never stalls PE

        def attn_group(qh, h, last_head):
            pair = h // 2
            # 4 accumulation slices share a PSUM bank and a matmul with
            # start=True zeroes the WHOLE bank, so pre-zero via DVE and
            # accumulate with start=False on every PV matmul.
            pva = pvp.tile([128, 4, 65], F32, tag="pva")
            pvb = pvp.tile([128, 4, 65], F32, tag="pvb")
            nc.vector.memset(pva[:], 0.0)
            nc.vector.memset(pvb[:], 0.0)
            ets = {}

            def emit_pv(kb):
                et = ets.pop(kb)
                for t in range(8):
                    dst = pva if t < 4 else pvb
                    nc.tensor.matmul(
                        dst[:, t % 4, :],
                        lhsT=et[:, t * 128:(t + 1) * 128],
                        rhs=v_sb[:, kb, h, :],
                        start=False, stop=(kb == 15),
                        skip_group_check=True)

            for kb in range(16):
                ps = scp.tile([128, 1024], F32, tag="sc")
                for j in range(2):
                    q0 = qh * 1024 + j * 512
                    nc.tensor.matmul(
                        ps[:, j * 512:(j + 1) * 512],
                        lhsT=ktz[h][:, kb * 128:(kb + 1) * 128],
                        rhs=qt[pair][:, q0:q0 + 512],
                        start=True, stop=True)
                et = epool.tile([128, 1024], BF16)
                nc.scalar.activation(et[:], ps[:], EXP,
                                     bias=mb[:, kb:kb + 1], scale=0.125)
                ets[kb] = et
                emit_proj_step()
                if kb >= LAG:
                    emit_pv(kb - LAG)
            for kb in range(16 - LAG, 16):
                emit_pv(kb)
            rca = small.tile([128, 4, 1], F32, tag="rca")
            rcb = small.tile([128, 4, 1], F32, tag="rcb")
            nc.vector.reciprocal(rca[:], pva[:, :, 64:65])
            nc.vector.reciprocal(rcb[:], pvb[:, :, 64:65])
            for t in range(8):
                src = pva if t < 4 else pvb
                rc = rca if t < 4 else rcb
                m = qh * 8 + t
                nc.vector.tensor_scalar_mul(
                    ost[:, m, h * 64:h * 64 + 64],
                    src[:, t % 4, 0:64], rc[:, t % 4, :])
                if last_head:
                    nc.sync.dma_start(out_d[m * 128:(m + 1) * 128, :],
                                      ost[:, m, :])

        # pair-0 groups first (pair-1 projections interleave 1 matmul/iter)
        groups = [(0, 0), (0, 1), (1, 0), (1, 1),
                  (0, 2), (0, 3), (1, 2), (1, 3)]
        for qh, h in groups:
            attn_group(qh, h, last_head=(h == 3))
    nc.compile()
    return nc


def _host_prep(x, attention_mask, Wq, Wk, Wv):
    x = np.asarray(x, dtype=np.float32)
    mask = np.asarray(attention_mask)
    Wq = np.asarray(Wq, dtype=np.float32)
    Wk = np.asarray(Wk, dtype=np.float32)
    Wv = np.asarray(Wv, dtype=np.float32)
    bf16 = ml_dtypes.bfloat16

    # rope fold: c_eff[b, d] = cos(b*th[d%32]) + sign(d)*sin(b*th[d%32])
    j = np.arange(0, HD, 2, dtype=np.float64) / HD          # [32]
    theta = 1.0 / (10000.0 ** j)                            # [32]
    dd = np.arange(HD)
    sign = np.where(dd < 32, 1.0, -1.0)

    def wlayout(wt_cols):  # [1024(k), 256] -> [128, 8*256] (partition-major)
        return np.ascontiguousarray(
            wt_cols.reshape(8, 128, 256).transpose(1, 0, 2).reshape(128, 8 * 256))

    in_maps = []
    wvt_full = np.ascontiguousarray(Wv.T).astype(bf16)      # [1024,1024]
    for b in range(B):
        ang = b * theta                                     # [32]
        ce = np.cos(ang[dd % 32]) + sign * np.sin(ang[dd % 32])  # [64]
        ccol = np.tile(ce, H).astype(np.float32)            # [1024]
        wqt_full = np.ascontiguousarray((Wq * ccol[:, None]).T).astype(bf16)
        wkt_full = np.ascontiguousarray((Wk * ccol[:, None]).T).astype(bf16)
        xtT = np.ascontiguousarray(x[b].T).astype(bf16)     # [1024, 2048]
        # [16 pieces, 128, 1024]: piece half*8+kc = xtT[kc-chunk, s-half]
        xt = np.ascontiguousarray(
            xtT.reshape(8, 128, 2, 1024).transpose(2, 0, 1, 3)).reshape(
            16, 128, 1024)
        maskb = np.ascontiguousarray(
            ((mask[b].astype(np.float32) - 1.0) * 30000.0).reshape(16, 128).T)
        for g in range(4):
            cols = slice(g * 256, (g + 1) * 256)
            in_maps.append({
                "xt": xt,
                "wqt": wlayout(wqt_full[:, cols]),
                "wkt": wlayout(wkt_full[:, cols]),
                "wvt": wlayout(wvt_full[:, cols]),
                "maskb": maskb,
            })
    return in_maps


def _get_nc():
    if "nc" not in _CACHE:
        _CACHE["nc"] = _build_nc()
    return _CACHE["nc"]


def kernel(x, attention_mask, Wq, Wk, Wv, **extra_kwargs):
    nc = _get_nc()
    in_maps = _host_prep(x, attention_mask, Wq, Wk, Wv)
    res = run_bass_kernel_spmd(nc, in_maps, list(range(NCORES))).results
    out = np.empty((B, S, D), dtype=np.float32)
    for c in range(NCORES):
        b, g = divmod(c, 4)
        out[b, :, g * 256:(g + 1) * 256] = res[c]["out"]
    return out


# revision 28
# speedup vs baseline: 1.1285x; 1.0039x over previous
"""Grouped self-attention (B=2, S=2048, D=1024, H=16, hd=64) on 8 trn2 cores.

Sharding: core c = b*4 + g handles batch b, heads [4g, 4g+4).

Key simplification: the reference's RoPE indexes its cos/sin cache by the
BATCH dim and uses neg_half = [t_first, -t_second], so rope(t)[b,s,h,d] =
t * (cos(b*th[d%32]) + sign(d)*sin(b*th[d%32])) — a pure per-(b,d) scale
that folds into rows of Wq/Wk on the host. The device kernel is then just
QKV projection + softmax attention.

v5 design (evolved via NTFF profiling):
- all matmul operands bf16 (fp32r streams at ~half rate), PSUM accum f32.
- phase 2 is ACT-bound: 128 exp tiles [128,1024] at ~1.1us each, ACT
  saturated back-to-back. Everything else aims to start that pipeline
  early and keep it unbroken.
- K stored as 4 zero-padded per-head tiles ktz[h] (real rows in the
  head's pair slot, zeros elsewhere) so scores matmuls run with a full
  128 contraction (64-contraction matmuls measured ~2.5x slower);
  Q keeps the packed pair layout (padded K rows zero the other head).
- PV: out[q,d] = et_chunk.T @ V_aug (ones column = softmax denominator).
  4 accumulation slices share a PSUM bank and start=True zeroes a whole
  bank, so banks are pre-zeroed via DVE memset and PV accumulates with
  start=False.
- DMA uses fat lines only (>=4KB per partition): xt as 8 kc-chunks of
  [128,2048], each weight matrix as a single [128,2048] transfer
  (small-line layouts measured at ~130GB/s vs ~290GB/s).
- the kt0 projection is 4-way chunk-interleaved (4 concurrent PSUM bank
  accumulators) so it tracks DMA arrival and finishes with the last xt
  chunk; first exp fires right after qt0 + V.
- ACT does exp only; PSUM reads (copies/normalize) on DVE; ktz padding
  memsets on GpSimd; output DMA'd per 128-row tile as its last head
  normalizes, hiding the writeback tail.
"""

import numpy as np
from contextlib import ExitStack

import ml_dtypes
import concourse.bass as bass
import concourse.bacc as bacc
import concourse.tile as tile
from concourse import mybir
from concourse.bass_utils import run_bass_kernel_spmd

F32 = mybir.dt.float32
BF16 = mybir.dt.bfloat16
EXP = mybir.ActivationFunctionType.Exp

B, S, D, H, HD = 2, 2048, 1024, 16, 64
NCORES = 8

_CACHE = {}


def _build_nc():
    nc = bacc.Bacc("TRN2", target_bir_lowering=False, debug=False)
    # xt piece half*8+kc = x[b].T[kc*128:(kc+1)*128, half*1024:(half+1)*1024]
    # (2KB lines measured at ~357GB/s; s-half-major order lets projections
    # and V start before the full tensor lands)
    xt_d = nc.declare_dram_parameter("xt", [16, 128, 1024], BF16, isOutput=False)
    # weights laid out [128, 8, 256] so one fat DMA fills the SBUF tile
    wqt_d = nc.declare_dram_parameter("wqt", [128, 8 * 256], BF16, isOutput=False)
    wkt_d = nc.declare_dram_parameter("wkt", [128, 8 * 256], BF16, isOutput=False)
    wvt_d = nc.declare_dram_parameter("wvt", [128, 8 * 256], BF16, isOutput=False)
    mb_d = nc.declare_dram_parameter("maskb", [128, 16], F32, isOutput=False)
    out_d = nc.declare_dram_parameter("out", [S, 256], F32, isOutput=True)

    with tile.TileContext(nc) as tc, ExitStack() as ctx:
        const = ctx.enter_context(tc.tile_pool(name="const", bufs=1))
        xpool = ctx.enter_context(tc.tile_pool(name="x", bufs=1))
        wpool = ctx.enter_context(tc.tile_pool(name="w", bufs=1))
        qkpool = ctx.enter_context(tc.tile_pool(name="qk", bufs=1))
        vpool = ctx.enter_context(tc.tile_pool(name="v", bufs=1))
        opool = ctx.enter_context(tc.tile_pool(name="o", bufs=1))
        epool = ctx.enter_context(tc.tile_pool(name="et", bufs=6))
        small = ctx.enter_context(tc.tile_pool(name="small", bufs=4))
        scp = ctx.enter_context(tc.tile_pool(name="scp", bufs=2, space="PSUM"))
        pvp = ctx.enter_context(tc.tile_pool(name="pvp", bufs=1, space="PSUM"))
        pjp = ctx.enter_context(tc.tile_pool(name="pjp", bufs=2, space="PSUM"))

        mb = const.tile([128, 16], F32)
        nc.sync.dma_start(mb[:], mb_d[:])
        wk = wpool.tile([128, 8 * 256], BF16, tag="wk")
        wq = wpool.tile([128, 8 * 256], BF16, tag="wq")
        wv = wpool.tile([128, 8 * 256], BF16, tag="wv")
        nc.sync.dma_start(wk[:], wkt_d[:])
        nc.sync.dma_start(wq[:], wqt_d[:])
        xt = xpool.tile([128, 8 * S], BF16)
        for kc in range(8):
            nc.sync.dma_start(xt[:, kc * S: kc * S + 1024], xt_d[kc])
        nc.sync.dma_start(wv[:], wvt_d[:])
        for kc in range(8):
            nc.sync.dma_start(xt[:, kc * S + 1024: kc * S + 2048], xt_d[8 + kc])

        qt = [qkpool.tile([128, S], BF16, tag=f"qt{p}", name=f"qt{p}")
              for p in range(2)]
        ktz = [qkpool.tile([128, S], BF16, tag=f"ktz{h}", name=f"ktz{h}")
               for h in range(4)]
        # zero the padding rows once (GpSimd: SBUF-only engine, keeps DVE free)
        for h in range(4):
            lo, hi = (64, 128) if h % 2 == 0 else (0, 64)
            nc.gpsimd.memset(ktz[h][lo:hi, :], 0.0)

        v_sb = vpool.tile([128, 16, 4, 65], BF16)
        nc.gpsimd.memset(v_sb[:, :, :, 64:65], 1.0)

        # prefix copies alternate DVE / ACT (ACT is idle until the first exp)
        _ceng = [0]

        def copy_eng():
            _ceng[0] ^= 1
            return nc.vector if _ceng[0] else nc.scalar

        def prefix_copy(dst, src):
            eng = copy_eng()
            if eng is nc.vector:
                eng.tensor_copy(dst, src)
            else:
                eng.copy(dst, src)

        def k_copies(pair, nb, ps, eng=None):
            sl = slice(nb * 512, (nb + 1) * 512)
            prefix_copy(ktz[2 * pair][0:64, sl], ps[0:64, 0:512])
            prefix_copy(ktz[2 * pair + 1][64:128, sl], ps[64:128, 0:512])

        def proj_v(m):
            pv = scp.tile([128, 1024], F32, tag="sc")
            for kc in range(8):
                nc.tensor.matmul(
                    pv[:, 0:256],
                    lhsT=xt[:, kc * S + m * 128: kc * S + m * 128 + 128],
                    rhs=wv[:, kc * 256:(kc + 1) * 256],
                    start=(kc == 0), stop=(kc == 7))
            prefix_copy(v_sb[:, m, :, 0:64], pv[:, 0:256])

        # ---- phase 1 ----
        # Per s-half: kt0 and qt0 nb-groups chunk-interleaved across 4 PSUM
        # banks (2 sc tiles x 2 bank-halves) tracking xt DMA arrival, then
        # that half's V while the next half streams in.
        def qk_half(half):
            nb0 = 2 * half
            t1 = scp.tile([128, 1024], F32, tag="sc")
            t2 = scp.tile([128, 1024], F32, tag="sc")
            slots = [t1[:, 0:512], t1[:, 512:1024],
                     t2[:, 0:512], t2[:, 512:1024]]
            for kc in range(8):
                for i, (wtile, nb) in enumerate(
                        ((wk, nb0), (wk, nb0 + 1), (wq, nb0), (wq, nb0 + 1))):
                    nc.tensor.matmul(
                        slots[i],
                        lhsT=wtile[:, kc * 256: kc * 256 + 128],
                        rhs=xt[:, kc * S + nb * 512: kc * S + nb * 512 + 512],
                        start=(kc == 0), stop=(kc == 7))
            for i, nb in ((0, nb0), (1, nb0 + 1)):
                sl = slice(nb * 512, (nb + 1) * 512)
                prefix_copy(ktz[0][0:64, sl], slots[i][0:64, :])
                prefix_copy(ktz[1][64:128, sl], slots[i][64:128, :])
            for i, nb in ((2, nb0), (3, nb0 + 1)):
                prefix_copy(qt[0][:, nb * 512:(nb + 1) * 512], slots[i][:, :])

        # group00's scores only need ktz0 (all s) + qt0 nb0-1, and with the
        # lagged PV only v_sb chunks a few iterations ahead. So the prefix is:
        # qk_half(0), V, then just kt0 nb2-3; qt0 nb2-3 and all pair-1
        # projections stream into phase 2 one matmul per kb iteration.
        qk_half(0)
        for m in range(16):
            proj_v(m)
        tk = scp.tile([128, 1024], F32, tag="sc")
        kslots = [tk[:, 0:512], tk[:, 512:1024]]
        for kc in range(8):
            for i, nb in ((0, 2), (1, 3)):
                nc.tensor.matmul(
                    kslots[i],
                    lhsT=wk[:, kc * 256: kc * 256 + 128],
                    rhs=xt[:, kc * S + nb * 512: kc * S + nb * 512 + 512],
                    start=(kc == 0), stop=(kc == 7))
        for i, nb in ((0, 2), (1, 3)):
            sl = slice(nb * 512, (nb + 1) * 512)
            prefix_copy(ktz[0][0:64, sl], kslots[i][0:64, :])
            prefix_copy(ktz[1][64:128, sl], kslots[i][64:128, :])

        # deferred projections: ordered so each lands just before its first
        # consumer group (checked against 1 step/iter + 2 steps/iter in g0)
        pieces = [(wq, 0, 2, "q0"), (wq, 0, 3, "q0"),
                  (wk, 1, 0, "k1"), (wk, 1, 1, "k1"),
                  (wq, 1, 0, "q1"), (wq, 1, 1, "q1"),
                  (wk, 1, 2, "k1"), (wk, 1, 3, "k1"),
                  (wq, 1, 2, "q1"), (wq, 1, 3, "q1")]
        pstate = {"pi": 0, "kc": 0, "ps": None}

        def emit_proj_step():
            if pstate["pi"] >= len(pieces):
                return
            wtile, pair, nb, kind = pieces[pstate["pi"]]
            kc = pstate["kc"]
            if kc == 0:
                pstate["ps"] = pjp.tile([128, 512], F32, tag="pj",
                                        name=f"pj{pstate['pi']}")
            lo = kc * 256 + pair * 128
            nc.tensor.matmul(
                pstate["ps"][:],
                lhsT=wtile[:, lo:lo + 128],
                rhs=xt[:, kc * S + nb * 512: kc * S + nb * 512 + 512],
                start=(kc == 0), stop=(kc == 7))
            pstate["kc"] += 1
            if pstate["kc"] == 8:
                ps = pstate["ps"]
                sl = slice(nb * 512, (nb + 1) * 512)
                if kind == "k1":
                    nc.vector.tensor_copy(ktz[2][0:64, sl], ps[0:64, :])
                    nc.vector.tensor_copy(ktz[3][64:128, sl], ps[64:128, :])
                elif kind == "q1":
                    nc.vector.tensor_copy(qt[1][:, sl], ps[:, :])
                else:
                    nc.vector.tensor_copy(qt[0][:, sl], ps[:, :])
                pstate["kc"] = 0
                pstate["pi"] += 1

        # ---- phase 2 ----
        ost = opool.tile([128, 16, 256], F32)

        LAG = 3  # PV trails exp by 3 iterations so pvp bufs=1 never stalls PE

        def attn_group(qh, h, last_head, proj_steps=1):
            pair = h // 2
            # 4 accumulation slices share a PSUM bank and a matmul with
            # start=True zeroes the WHOLE bank, so pre-zero via DVE and
            # accumulate with start=False on every PV matmul.
            pva = pvp.tile([128, 4, 65], F32, tag="pva")
            pvb = pvp.tile([128, 4, 65], F32, tag="pvb")
            nc.vector.memset(pva[:], 0.0)
            nc.vector.memset(pvb[:], 0.0)
            ets = {}

            def emit_pv(kb):
                et = ets.pop(kb)
                for t in range(8):
                    dst = pva if t < 4 else pvb
                    nc.tensor.matmul(
                        dst[:, t % 4, :],
                        lhsT=et[:, t * 128:(t + 1) * 128],
                        rhs=v_sb[:, kb, h, :],
                        start=False, stop=(kb == 15),
                        skip_group_check=True)

            for kb in range(16):
                ps = scp.tile([128, 1024], F32, tag="sc")
                for j in range(2):
                    q0 = qh * 1024 + j * 512
                    nc.tensor.matmul(
                        ps[:, j * 512:(j + 1) * 512],
                        lhsT=ktz[h][:, kb * 128:(kb + 1) * 128],
                        rhs=qt[pair][:, q0:q0 + 512],
                        start=True, stop=True)
                et = epool.tile([128, 1024], BF16)
                nc.scalar.activation(et[:], ps[:], EXP,
                                     bias=mb[:, kb:kb + 1], scale=0.125)
                ets[kb] = et
                for _ in range(proj_steps):
                    emit_proj_step()
                if kb >= LAG:
                    emit_pv(kb - LAG)
            for kb in range(16 - LAG, 16):
                emit_pv(kb)
            rca = small.tile([128, 4, 1], F32, tag="rca")
            rcb = small.tile([128, 4, 1], F32, tag="rcb")
            nc.vector.reciprocal(rca[:], pva[:, :, 64:65])
            nc.vector.reciprocal(rcb[:], pvb[:, :, 64:65])
            for t in range(8):
                src = pva if t < 4 else pvb
                rc = rca if t < 4 else rcb
                m = qh * 8 + t
                nc.vector.tensor_scalar_mul(
                    ost[:, m, h * 64:h * 64 + 64],
                    src[:, t % 4, 0:64], rc[:, t % 4, :])
                if last_head:
                    nc.sync.dma_start(out_d[m * 128:(m + 1) * 128, :],
                                      ost[:, m, :])

        # pair-0 groups first (deferred projections interleave per-iteration)
        groups = [(0, 0), (0, 1), (1, 0), (1, 1),
                  (0, 2), (0, 3), (1, 2), (1, 3)]
        for gi, (qh, h) in enumerate(groups):
            attn_group(qh, h, last_head=(h == 3), proj_steps=(2 if gi == 0 else 1))
    nc.compile()
    return nc


def _host_prep(x, attention_mask, Wq, Wk, Wv):
    x = np.asarray(x, dtype=np.float32)
    mask = np.asarray(attention_mask)
    Wq = np.asarray(Wq, dtype=np.float32)
    Wk = np.asarray(Wk, dtype=np.float32)
    Wv = np.asarray(Wv, dtype=np.float32)
    bf16 = ml_dtypes.bfloat16

    # rope fold: c_eff[b, d] = cos(b*th[d%32]) + sign(d)*sin(b*th[d%32])
    j = np.arange(0, HD, 2, dtype=np.float64) / HD          # [32]
    theta = 1.0 / (10000.0 ** j)                            # [32]
    dd = np.arange(HD)
    sign = np.where(dd < 32, 1.0, -1.0)

    def wlayout(wt_cols):  # [1024(k), 256] -> [128, 8*256] (partition-major)
        return np.ascontiguousarray(
            wt_cols.reshape(8, 128, 256).transpose(1, 0, 2).reshape(128, 8 * 256))

    in_maps = []
    wvt_full = np.ascontiguousarray(Wv.T).astype(bf16)      # [1024,1024]
    for b in range(B):
        ang = b * theta                                     # [32]
        ce = np.cos(ang[dd % 32]) + sign * np.sin(ang[dd % 32])  # [64]
        ccol = np.tile(ce, H).astype(np.float32)            # [1024]
        wqt_full = np.ascontiguousarray((Wq * ccol[:, None]).T).astype(bf16)
        wkt_full = np.ascontiguousarray((Wk * ccol[:, None]).T).astype(bf16)
        xtT = np.ascontiguousarray(x[b].T).astype(bf16)     # [1024, 2048]
        # [16 pieces, 128, 1024]: piece half*8+kc = xtT[kc-chunk, s-half]
        xt = np.ascontiguousarray(
            xtT.reshape(8, 128, 2, 1024).transpose(2, 0, 1, 3)).reshape(
            16, 128, 1024)
        maskb = np.ascontiguousarray(
            ((mask[b].astype(np.float32) - 1.0) * 30000.0).reshape(16, 128).T)
        for g in range(4):
            cols = slice(g * 256, (g + 1) * 256)
            in_maps.append({
                "xt": xt,
                "wqt": wlayout(wqt_full[:, cols]),
                "wkt": wlayout(wkt_full[:, cols]),
                "wvt": wlayout(wvt_full[:, cols]),
                "maskb": maskb,
            })
    return in_maps


def _get_nc():
    if "nc" not in _CACHE:
        _CACHE["nc"] = _build_nc()
    return _CACHE["nc"]


def kernel(x, attention_mask, Wq, Wk, Wv, **extra_kwargs):
    nc = _get_nc()
    in_maps = _host_prep(x, attention_mask, Wq, Wk, Wv)
    res = run_bass_kernel_spmd(nc, in_maps, list(range(NCORES))).results
    out = np.empty((B, S, D), dtype=np.float32)
    for c in range(NCORES):
        b, g = divmod(c, 4)
        out[b, :, g * 256:(g + 1) * 256] = res[c]["out"]
    return out


# revision 31
# speedup vs baseline: 1.1700x; 1.0368x over previous
"""Grouped self-attention (B=2, S=2048, D=1024, H=16, hd=64) on 8 trn2 cores.

Sharding: core c = b*4 + g handles batch b, heads [4g, 4g+4).

Key simplification: the reference's RoPE indexes its cos/sin cache by the
BATCH dim and uses neg_half = [t_first, -t_second], so rope(t)[b,s,h,d] =
t * (cos(b*th[d%32]) + sign(d)*sin(b*th[d%32])) — a pure per-(b,d) scale
that folds into rows of Wq/Wk on the host. The device kernel is then just
QKV projection + softmax attention.

v5 design (evolved via NTFF profiling):
- all matmul operands bf16 (fp32r streams at ~half rate), PSUM accum f32.
- phase 2 is ACT-bound: 128 exp tiles [128,1024] at ~1.1us each, ACT
  saturated back-to-back. Everything else aims to start that pipeline
  early and keep it unbroken.
- K stored as 4 zero-padded per-head tiles ktz[h] (real rows in the
  head's pair slot, zeros elsewhere) so scores matmuls run with a full
  128 contraction (64-contraction matmuls measured ~2.5x slower);
  Q keeps the packed pair layout (padded K rows zero the other head).
- PV: out[q,d] = et_chunk.T @ V_aug (ones column = softmax denominator).
  4 accumulation slices share a PSUM bank and start=True zeroes a whole
  bank, so banks are pre-zeroed via DVE memset and PV accumulates with
  start=False.
- DMA uses fat lines only (>=4KB per partition): xt as 8 kc-chunks of
  [128,2048], each weight matrix as a single [128,2048] transfer
  (small-line layouts measured at ~130GB/s vs ~290GB/s).
- the kt0 projection is 4-way chunk-interleaved (4 concurrent PSUM bank
  accumulators) so it tracks DMA arrival and finishes with the last xt
  chunk; first exp fires right after qt0 + V.
- ACT does exp only; PSUM reads (copies/normalize) on DVE; ktz padding
  memsets on GpSimd; output DMA'd per 128-row tile as its last head
  normalizes, hiding the writeback tail.
"""

import numpy as np
from contextlib import ExitStack

import ml_dtypes
import concourse.bass as bass
import concourse.bacc as bacc
import concourse.tile as tile
from concourse import mybir
from concourse.bass_utils import run_bass_kernel_spmd

F32 = mybir.dt.float32
BF16 = mybir.dt.bfloat16
EXP = mybir.ActivationFunctionType.Exp

B, S, D, H, HD = 2, 2048, 1024, 16, 64
NCORES = 8

_CACHE = {}


def _build_nc():
    nc = bacc.Bacc("TRN2", target_bir_lowering=False, debug=False)
    # xt piece half*8+kc = x[b].T[kc*128:(kc+1)*128, half*1024:(half+1)*1024]
    # (2KB lines measured at ~357GB/s; s-half-major order lets projections
    # and V start before the full tensor lands)
    xt_d = nc.declare_dram_parameter("xt", [16, 128, 1024], BF16, isOutput=False)
    # weights laid out [128, 8, 256] so one fat DMA fills the SBUF tile
    wqt_d = nc.declare_dram_parameter("wqt", [128, 8 * 256], BF16, isOutput=False)
    wkt_d = nc.declare_dram_parameter("wkt", [128, 8 * 256], BF16, isOutput=False)
    wvt_d = nc.declare_dram_parameter("wvt", [128, 8 * 256], BF16, isOutput=False)
    mb_d = nc.declare_dram_parameter("maskb", [128, 16], F32, isOutput=False)
    out_d = nc.declare_dram_parameter("out", [S, 256], F32, isOutput=True)

    with tile.TileContext(nc) as tc, ExitStack() as ctx:
        const = ctx.enter_context(tc.tile_pool(name="const", bufs=1))
        xpool = ctx.enter_context(tc.tile_pool(name="x", bufs=1))
        wpool = ctx.enter_context(tc.tile_pool(name="w", bufs=1))
        qkpool = ctx.enter_context(tc.tile_pool(name="qk", bufs=1))
        vpool = ctx.enter_context(tc.tile_pool(name="v", bufs=1))
        opool = ctx.enter_context(tc.tile_pool(name="o", bufs=1))
        epool = ctx.enter_context(tc.tile_pool(name="et", bufs=6))
        small = ctx.enter_context(tc.tile_pool(name="small", bufs=4))
        scp = ctx.enter_context(tc.tile_pool(name="scp", bufs=2, space="PSUM"))
        pvp = ctx.enter_context(tc.tile_pool(name="pvp", bufs=1, space="PSUM"))
        pjp = ctx.enter_context(tc.tile_pool(name="pjp", bufs=2, space="PSUM"))

        mb = const.tile([128, 16], F32)
        nc.sync.dma_start(mb[:], mb_d[:])
        wk = wpool.tile([128, 8 * 256], BF16, tag="wk")
        wq = wpool.tile([128, 8 * 256], BF16, tag="wq")
        wv = wpool.tile([128, 8 * 256], BF16, tag="wv")
        nc.sync.dma_start(wk[:], wkt_d[:])
        nc.sync.dma_start(wq[:], wqt_d[:])
        xt = xpool.tile([128, 8 * S], BF16)
        for kc in range(8):
            nc.sync.dma_start(xt[:, kc * S: kc * S + 1024], xt_d[kc])
        nc.sync.dma_start(wv[:], wvt_d[:])
        for kc in range(8):
            nc.sync.dma_start(xt[:, kc * S + 1024: kc * S + 2048], xt_d[8 + kc])

        qt = [qkpool.tile([128, S], BF16, tag=f"qt{p}", name=f"qt{p}")
              for p in range(2)]
        ktz = [qkpool.tile([128, S], BF16, tag=f"ktz{h}", name=f"ktz{h}")
               for h in range(4)]
        # zero the padding rows once (GpSimd: SBUF-only engine, keeps DVE free)
        for h in range(4):
            lo, hi = (64, 128) if h % 2 == 0 else (0, 64)
            nc.gpsimd.memset(ktz[h][lo:hi, :], 0.0)

        v_sb = vpool.tile([128, 16, 4, 65], BF16)
        nc.gpsimd.memset(v_sb[:, :, :, 64:65], 1.0)

        # prefix copies alternate DVE / ACT (ACT is idle until the first exp)
        _ceng = [0]

        def copy_eng():
            _ceng[0] ^= 1
            return nc.vector if _ceng[0] else nc.scalar

        def prefix_copy(dst, src):
            eng = copy_eng()
            if eng is nc.vector:
                eng.tensor_copy(dst, src)
            else:
                eng.copy(dst, src)

        def k_copies(pair, nb, ps, eng=None):
            sl = slice(nb * 512, (nb + 1) * 512)
            prefix_copy(ktz[2 * pair][0:64, sl], ps[0:64, 0:512])
            prefix_copy(ktz[2 * pair + 1][64:128, sl], ps[64:128, 0:512])

        def proj_v(m):
            pv = scp.tile([128, 1024], F32, tag="sc")
            for kc in range(8):
                nc.tensor.matmul(
                    pv[:, 0:256],
                    lhsT=xt[:, kc * S + m * 128: kc * S + m * 128 + 128],
                    rhs=wv[:, kc * 256:(kc + 1) * 256],
                    start=(kc == 0), stop=(kc == 7))
            prefix_copy(v_sb[:, m, :, 0:64], pv[:, 0:256])

        # ---- phase 1 ----
        # Per s-half: kt0 and qt0 nb-groups chunk-interleaved across 4 PSUM
        # banks (2 sc tiles x 2 bank-halves) tracking xt DMA arrival, then
        # that half's V while the next half streams in.
        def qk_half(half):
            nb0 = 2 * half
            t1 = scp.tile([128, 1024], F32, tag="sc")
            t2 = scp.tile([128, 1024], F32, tag="sc")
            slots = [t1[:, 0:512], t1[:, 512:1024],
                     t2[:, 0:512], t2[:, 512:1024]]
            for kc in range(8):
                for i, (wtile, nb) in enumerate(
                        ((wk, nb0), (wk, nb0 + 1), (wq, nb0), (wq, nb0 + 1))):
                    nc.tensor.matmul(
                        slots[i],
                        lhsT=wtile[:, kc * 256: kc * 256 + 128],
                        rhs=xt[:, kc * S + nb * 512: kc * S + nb * 512 + 512],
                        start=(kc == 0), stop=(kc == 7))
            for i, nb in ((0, nb0), (1, nb0 + 1)):
                sl = slice(nb * 512, (nb + 1) * 512)
                prefix_copy(ktz[0][0:64, sl], slots[i][0:64, :])
                prefix_copy(ktz[1][64:128, sl], slots[i][64:128, :])
            for i, nb in ((2, nb0), (3, nb0 + 1)):
                prefix_copy(qt[0][:, nb * 512:(nb + 1) * 512], slots[i][:, :])

        # group00's scores only need ktz0 (all s) + qt0 nb0-1, and with the
        # lagged PV only v_sb chunks a few iterations ahead. So the prefix is:
        # qk_half(0), V, then just kt0 nb2-3; qt0 nb2-3 and all pair-1
        # projections stream into phase 2 one matmul per kb iteration.
        qk_half(0)
        for m in range(16):
            proj_v(m)

        # deferred projections: ordered so each lands just before its first
        # consumer (checked against 2 steps/iter in g0-g1, 1 after): kt0 nb2/3
        # feed g0's kb8/kb12 scores; kt1+qt1 feed group (0,2) at iter 64.
        pieces = [(wk, 0, 2, "k0"), (wk, 0, 3, "k0"),
                  (wq, 0, 2, "q0"), (wq, 0, 3, "q0"),
                  (wk, 1, 0, "k1"), (wk, 1, 1, "k1"),
                  (wk, 1, 2, "k1"), (wk, 1, 3, "k1"),
                  (wq, 1, 0, "q1"), (wq, 1, 1, "q1"),
                  (wq, 1, 2, "q1"), (wq, 1, 3, "q1")]
        pstate = {"pi": 0, "kc": 0, "ps": None}

        def emit_proj_step():
            if pstate["pi"] >= len(pieces):
                return
            wtile, pair, nb, kind = pieces[pstate["pi"]]
            kc = pstate["kc"]
            if kc == 0:
                pstate["ps"] = pjp.tile([128, 512], F32, tag="pj",
                                        name=f"pj{pstate['pi']}")
            lo = kc * 256 + pair * 128
            nc.tensor.matmul(
                pstate["ps"][:],
                lhsT=wtile[:, lo:lo + 128],
                rhs=xt[:, kc * S + nb * 512: kc * S + nb * 512 + 512],
                start=(kc == 0), stop=(kc == 7))
            pstate["kc"] += 1
            if pstate["kc"] == 8:
                ps = pstate["ps"]
                sl = slice(nb * 512, (nb + 1) * 512)
                if kind == "k0":
                    nc.vector.tensor_copy(ktz[0][0:64, sl], ps[0:64, :])
                    nc.vector.tensor_copy(ktz[1][64:128, sl], ps[64:128, :])
                elif kind == "k1":
                    nc.vector.tensor_copy(ktz[2][0:64, sl], ps[0:64, :])
                    nc.vector.tensor_copy(ktz[3][64:128, sl], ps[64:128, :])
                elif kind == "q1":
                    nc.vector.tensor_copy(qt[1][:, sl], ps[:, :])
                else:
                    nc.vector.tensor_copy(qt[0][:, sl], ps[:, :])
                pstate["kc"] = 0
                pstate["pi"] += 1

        # ---- phase 2 ----
        ost = opool.tile([128, 16, 256], F32)

        LAG = 3  # PV trails exp by 3 iterations so pvp bufs=1 never stalls PE

        def attn_group(qh, h, last_head, proj_steps=1):
            pair = h // 2
            # 4 accumulation slices share a PSUM bank and a matmul with
            # start=True zeroes the WHOLE bank, so pre-zero via DVE and
            # accumulate with start=False on every PV matmul.
            pva = pvp.tile([128, 4, 65], F32, tag="pva")
            pvb = pvp.tile([128, 4, 65], F32, tag="pvb")
            nc.vector.memset(pva[:], 0.0)
            nc.vector.memset(pvb[:], 0.0)
            ets = {}

            def emit_pv(kb):
                et = ets.pop(kb)
                for t in range(8):
                    dst = pva if t < 4 else pvb
                    nc.tensor.matmul(
                        dst[:, t % 4, :],
                        lhsT=et[:, t * 128:(t + 1) * 128],
                        rhs=v_sb[:, kb, h, :],
                        start=False, stop=(kb == 15),
                        skip_group_check=True)

            for kb in range(16):
                ps = scp.tile([128, 1024], F32, tag="sc")
                for j in range(2):
                    q0 = qh * 1024 + j * 512
                    nc.tensor.matmul(
                        ps[:, j * 512:(j + 1) * 512],
                        lhsT=ktz[h][:, kb * 128:(kb + 1) * 128],
                        rhs=qt[pair][:, q0:q0 + 512],
                        start=True, stop=True)
                et = epool.tile([128, 1024], BF16)
                nc.scalar.activation(et[:], ps[:], EXP,
                                     bias=mb[:, kb:kb + 1], scale=0.125)
                ets[kb] = et
                for _ in range(proj_steps):
                    emit_proj_step()
                if kb >= LAG:
                    emit_pv(kb - LAG)
            for kb in range(16 - LAG, 16):
                emit_pv(kb)
            rca = small.tile([128, 4, 1], F32, tag="rca")
            rcb = small.tile([128, 4, 1], F32, tag="rcb")
            nc.vector.reciprocal(rca[:], pva[:, :, 64:65])
            nc.vector.reciprocal(rcb[:], pvb[:, :, 64:65])
            for t in range(8):
                src = pva if t < 4 else pvb
                rc = rca if t < 4 else rcb
                m = qh * 8 + t
                nc.vector.tensor_scalar_mul(
                    ost[:, m, h * 64:h * 64 + 64],
                    src[:, t % 4, 0:64], rc[:, t % 4, :])
                if last_head:
                    nc.sync.dma_start(out_d[m * 128:(m + 1) * 128, :],
                                      ost[:, m, :])

        # pair-0 groups first (deferred projections interleave per-iteration)
        groups = [(0, 0), (0, 1), (1, 0), (1, 1),
                  (0, 2), (0, 3), (1, 2), (1, 3)]
        for gi, (qh, h) in enumerate(groups):
            attn_group(qh, h, last_head=(h == 3), proj_steps=(2 if gi < 2 else 1))
    nc.compile()
    return nc


def _host_prep(x, attention_mask, Wq, Wk, Wv):
    x = np.asarray(x, dtype=np.float32)
    mask = np.asarray(attention_mask)
    Wq = np.asarray(Wq, dtype=np.float32)
    Wk = np.asarray(Wk, dtype=np.float32)
    Wv = np.asarray(Wv, dtype=np.float32)
    bf16 = ml_dtypes.bfloat16

    # rope fold: c_eff[b, d] = cos(b*th[d%32]) + sign(d)*sin(b*th[d%32])
    j = np.arange(0, HD, 2, dtype=np.float64) / HD          # [32]
    theta = 1.0 / (10000.0 ** j)                            # [32]
    dd = np.arange(HD)
    sign = np.where(dd < 32, 1.0, -1.0)

    def wlayout(wt_cols):  # [1024(k), 256] -> [128, 8*256] (partition-major)
        return np.ascontiguousarray(
            wt_cols.reshape(8, 128, 256).transpose(1, 0, 2).reshape(128, 8 * 256))

    in_maps = []
    wvt_full = np.ascontiguousarray(Wv.T).astype(bf16)      # [1024,1024]
    for b in range(B):
        ang = b * theta                                     # [32]
        ce = np.cos(ang[dd % 32]) + sign * np.sin(ang[dd % 32])  # [64]
        ccol = np.tile(ce, H).astype(np.float32)            # [1024]
        wqt_full = np.ascontiguousarray((Wq * ccol[:, None]).T).astype(bf16)
        wkt_full = np.ascontiguousarray((Wk * ccol[:, None]).T).astype(bf16)
        xtT = np.ascontiguousarray(x[b].T).astype(bf16)     # [1024, 2048]
        # [16 pieces, 128, 1024]: piece half*8+kc = xtT[kc-chunk, s-half]
        xt = np.ascontiguousarray(
            xtT.reshape(8, 128, 2, 1024).transpose(2, 0, 1, 3)).reshape(
            16, 128, 1024)
        maskb = np.ascontiguousarray(
            ((mask[b].astype(np.float32) - 1.0) * 30000.0).reshape(16, 128).T)
        for g in range(4):
            cols = slice(g * 256, (g + 1) * 256)
            in_maps.append({
                "xt": xt,
                "wqt": wlayout(wqt_full[:, cols]),
                "wkt": wlayout(wkt_full[:, cols]),
                "wvt": wlayout(wvt_full[:, cols]),
                "maskb": maskb,
            })
    return in_maps


def _get_nc():
    if "nc" not in _CACHE:
        _CACHE["nc"] = _build_nc()
    return _CACHE["nc"]


def kernel(x, attention_mask, Wq, Wk, Wv, **extra_kwargs):
    nc = _get_nc()
    in_maps = _host_prep(x, attention_mask, Wq, Wk, Wv)
    res = run_bass_kernel_spmd(nc, in_maps, list(range(NCORES))).results
    out = np.empty((B, S, D), dtype=np.float32)
    for c in range(NCORES):
        b, g = divmod(c, 4)
        out[b, :, g * 256:(g + 1) * 256] = res[c]["out"]
    return out
